# revision 1
# baseline (speedup 1.0000x reference)
"""DeepseekV2 MLA attention (matrix-absorbed, causal MQA) on 8 TRN2 cores.

Tensor-parallel over heads: 2 heads per core. Per core:
  - q/kv projections from a host-pre-transposed x^T (bf16)
  - latent RMS-norm (kv_norm_w folded host-side into kc/vc)
  - scores computed in transposed layout S^T[t, s] so exp/mask/AV flow
    without transposing the attention matrix
  - softmax denominator via ones-vector matmul on PE; normalization deferred
    to the small out_v^T tensor (broadcast via gpsimd partition_broadcast)
  - o_proj over this core's 2 heads -> partial [S, HID], host sums 8 partials
"""

import os

import numpy as np
import ml_dtypes

# best-effort persistent compile cache (harmless if the PJRT plugin
# doesn't support executable serialization)
os.environ.setdefault("JAX_COMPILATION_CACHE_DIR", "/tmp/jax_cache")
os.environ.setdefault("JAX_PERSISTENT_CACHE_MIN_COMPILE_TIME_SECS", "1")

S, HID, H = 2048, 2048, 16
NOPE, ROPE, KVR, VH = 128, 64, 512, 128
SCALE = (NOPE + ROPE) ** -0.5
EPS = 1e-6
NCORES = 8
HPC = H // NCORES  # heads per core = 2

BF16 = ml_dtypes.bfloat16

_CACHE = {}


def _build_nc(s_len):
    import concourse.bass as bass
    import concourse.tile as tile
    from concourse import bacc, mybir
    from concourse.bass import ts, ds
    from concourse.tile_rust import add_dep_helper

    f32 = mybir.dt.float32
    bf16 = mybir.dt.bfloat16

    NB = s_len // 512   # 512-wide seq blocks
    NT = s_len // 128   # 128-wide seq tiles

    nc = bacc.Bacc()

    xt = nc.declare_dram_parameter("xt", [NB, 128, 16, 512], bf16, isOutput=False)
    # this core's own 256 x^T columns (for its 2 latent tiles)
    xp = nc.declare_dram_parameter("xp", [128, 16, 256], bf16, isOutput=False)
    wkv = nc.declare_dram_parameter("wkv", [128, 16, 576], bf16, isOutput=False)
    wqn = nc.declare_dram_parameter("wqn", [128, 16, 256], bf16, isOutput=False)
    wqp = nc.declare_dram_parameter("wqp", [128, 16, 128], bf16, isOutput=False)
    # kc^T chunks [k-part, kk, h, d] for the per-head key materialization
    kcp = nc.declare_dram_parameter("kcp", [128, 4, 2, 128], bf16,
                                    isOutput=False)
    vcp = nc.declare_dram_parameter("vcp", [128, 2, 4, 128], bf16, isOutput=False)
    wo = nc.declare_dram_parameter("wo", [128, 2, 2048], bf16, isOutput=False)
    # rope tables for this core's 2 tiles: [p, lt, 0:64]=cos, [, 64:128]=sin_eff
    # rope rows: [0:4] = block-0 tiles (same on all cores), [4:6] = this
    # core's two sharded tiles
    csl = nc.declare_dram_parameter("csl", [128, 6, 128], f32, isOutput=False)
    # transposed rope tables, duplicated across both 64-partition halves:
    # [p, 0, s]=cos^T, [p, 1, s]=sin_eff^T (p in 0:64 == p in 64:128)
    cst = nc.declare_dram_parameter("cst", [128, 2, s_len], bf16,
                                    isOutput=False)
    # bf16 partials: host sums 8 per-core partials in f64, so the only cost
    # is one bf16 rounding per partial (~0.3% on a 2% budget); halves the
    # output DMA traffic that gates the o_proj store pipeline
    out = nc.declare_dram_parameter("out", [s_len, HID], bf16, isOutput=True)

    with tile.TileContext(nc) as tc:
        with (
            tc.tile_pool(name="singles", bufs=1) as singles,
            tc.tile_pool(name="state", bufs=1) as state,
            tc.tile_pool(name="xpool", bufs=5) as xpool,
            tc.tile_pool(name="attn", bufs=21) as attnp,
            tc.tile_pool(name="work", bufs=2) as work,
            tc.tile_pool(name="scr", bufs=2) as scr,
            tc.tile_pool(name="stat", bufs=4) as statp,
            tc.tile_pool(name="outp", bufs=4) as outp,
            tc.tile_pool(name="dram", bufs=1, space="DRAM") as dram,
            tc.tile_pool(name="pmm", bufs=5, space="PSUM") as pmm,
            tc.tile_pool(name="psmall", bufs=2, space="PSUM") as psmall,
            tc.tile_pool(name="pden", bufs=1, space="PSUM") as pden,
        ):
            # ---- static weights; SP-queue loads ordered by first use ----
            wkv_sb = singles.tile([128, 16, 576], bf16)
            for c in range(4):
                nc.sync.dma_start(out=wkv_sb[:, ts(c, 4), :],
                                  in_=wkv[:, ts(c, 4), :])
            csl_sb = singles.tile([128, 6, 128], f32)
            nc.sync.dma_start(out=csl_sb, in_=csl[:])
            wqn_sb = singles.tile([128, 16, 256], bf16)
            for c in range(2):
                nc.sync.dma_start(out=wqn_sb[:, ts(c, 8), :],
                                  in_=wqn[:, ts(c, 8), :])
            kc_sb = singles.tile([128, 4, 2, 128], bf16)
            nc.sync.dma_start(out=kc_sb, in_=kcp[:])
            wqp_sb = singles.tile([128, 16, 128], bf16)
            nc.sync.dma_start(out=wqp_sb, in_=wqp[:])
            cst_sb = singles.tile([128, 2, s_len], bf16)
            nc.sync.dma_start(out=cst_sb, in_=cst[:])
            vc_sb = singles.tile([128, 2, 4, 128], bf16)
            nc.sync.dma_start(out=vc_sb, in_=vcp[:])
            wo_sb = singles.tile([128, 2, 2048], bf16)
            for c in range(2):
                nc.sync.dma_start(out=wo_sb[:, c, :], in_=wo[:, c, :])
            # DVE observes the csl/cst DMAs via single-wait touch ops; the
            # rope muls (PSUM + table inputs) can then carry only the PE
            # wait — DVE TensorTensor encodes at most one sync wait.
            csl_touch = singles.tile([128, 1], f32)
            csl_touch_inst = nc.vector.tensor_copy(csl_touch, csl_sb[:, 0, 0:1])
            cst_touch = singles.tile([128, 1], bf16)
            cst_touch_inst = nc.vector.tensor_copy(cst_touch, cst_sb[:, 0, 0:1])

            ones_bf = singles.tile([128, 128], bf16)
            nc.vector.memset(ones_bf, 1.0)
            eps_sb = singles.tile([128, 1], f32)
            nc.vector.memset(eps_sb, EPS)

            # ---- shared state (filled by the allgather) ----
            # per-head un-absorbed values lv_h = latent_norm @ vc_h [t, v]:
            # with only 2 heads per core this costs 6.8us to build but makes
            # AV a single-chunk contraction (at^T @ lv) instead of a 4-chunk
            # latent-space one plus a vc re-projection
            lv_sb = state.tile([128, 2, NT, 128], bf16)
            # per-head un-absorbed keys k_h^T[d, t] = kc_h @ latent^T: same
            # trick on the K side — scores contract over d=128 instead of
            # the 512-dim latent (1 matmul per tile instead of 4)
            khT_sb = state.tile([128, 2, NT, 128], bf16)
            # staging for block-0 local latent tiles (transpose source only)
            lnb = state.tile([128, 4, 512], bf16)
            latT_sb = state.tile([128, 4, s_len], bf16)  # latent_norm^T [k, t]
            # k_pe_rot^T duplicated in both partition halves so each head's
            # q_pe rows (base partition 0 / 64) pair with a matching lhsT
            kpeT_sb = state.tile([128, s_len], bf16)
            ovT_sb = state.tile([128, 2, s_len], bf16)   # out_v^T (normalized)

            # ---- kv tile worker: latent rms + k_pe rope for one t-tile
            def kv_tile(xcols, cs_ap, ln_out, kpeT_out, latT_out):
                p_lat = pmm.tile([128, 512], f32, tag="mm")
                p_pe = psmall.tile([128, 64], f32, tag="small")
                for kt in range(16):
                    nc.tensor.matmul(
                        p_lat, lhsT=xcols(kt), rhs=wkv_sb[:, kt, 0:512],
                        start=(kt == 0), stop=(kt == 15))
                    nc.tensor.matmul(
                        p_pe, lhsT=xcols(kt), rhs=wkv_sb[:, kt, 512:576],
                        start=(kt == 0), stop=(kt == 15))
                # rms over k (free dim)
                sqs = scr.tile([128, 512], f32, tag="scr")
                stats = statp.tile([128, 3], f32, tag="stat")
                nc.scalar.activation(
                    out=sqs, in_=p_lat,
                    func=mybir.ActivationFunctionType.Square,
                    accum_out=stats[:, 0:1])
                nc.scalar.activation(
                    out=stats[:, 1:2], in_=stats[:, 0:1],
                    func=mybir.ActivationFunctionType.Sqrt,
                    scale=1.0 / KVR, bias=eps_sb)
                nc.vector.reciprocal(stats[:, 2:3], stats[:, 1:2])
                nc.vector.tensor_scalar_mul(
                    out=ln_out, in0=p_lat, scalar1=stats[:, 2:3])
                # k_pe rope (fp32 from psum)
                t1 = statp.tile([128, 64], f32, tag="r1")
                t2 = statp.tile([128, 64], f32, tag="r2")
                i1 = nc.vector.tensor_mul(t1, p_pe, cs_ap[:, 0:64])
                i2 = nc.vector.tensor_mul(
                    t2[:, 0:32], p_pe[:, 32:64], cs_ap[:, 64:96])
                i3 = nc.vector.tensor_mul(
                    t2[:, 32:64], p_pe[:, 0:32], cs_ap[:, 96:128])
                for ii in (i1, i2, i3):
                    add_dep_helper(ii.ins, csl_touch_inst.ins, sync=False,
                                   reason="csl first-touch order")
                kpe_rot = statp.tile([128, 128], bf16, tag="kprot")
                nc.vector.tensor_add(kpe_rot[:, 0:64], t1, t2)
                nc.vector.tensor_add(kpe_rot[:, 64:128], t1, t2)
                # ACT HWDGE queue: keeps the SP queue free for the
                # gather-critical scatter (FIFO head-of-line matters)
                nc.scalar.dma_start_transpose(out=kpeT_out, in_=kpe_rot)
                if latT_out is not None:
                    nc.scalar.dma_start_transpose(out=latT_out, in_=ln_out)

            # ---- phase 0: hybrid kv sharding ----
            # Block 0's four latent tiles are computed locally on EVERY
            # core (fills the DMA-bound ramp and unblocks block-0 attention
            # with no gather dependency). Tiles 4..15 are sharded: cores
            # 0..5 each compute two of them and an AllGather distributes
            # the results while blocks 0-1 of q projections/attention run.
            # Cores 6,7 compute dummy tiles whose gather slots are ignored.
            xp_sb = singles.tile([128, 16, 256], bf16)
            nc.gpsimd.dma_start(out=xp_sb[:, 0:8, :], in_=xp[:, 0:8, :])
            nc.scalar.dma_start(out=xp_sb[:, 8:16, :], in_=xp[:, 8:16, :])
            a0chunks = []
            for c in range(4):
                xc = xpool.tile([128, 4, 512], bf16, tag="xt")
                eng = nc.scalar if c % 2 == 1 else nc.gpsimd
                eng.dma_start(out=xc, in_=xt[0, :, ts(c, 4), :])
                a0chunks.append(xc)

            # this core's 2 sharded tiles (from xp) -> bounce -> gather
            lnl = singles.tile([128, 2, 512], bf16)
            latTl = singles.tile([128, 4, 256], bf16)
            kplT = singles.tile([128, 256], bf16)
            for lt in range(2):
                kv_tile(
                    (lambda kt, lt=lt: xp_sb[:, kt, ts(lt, 128)]),
                    csl_sb[:, 4 + lt, :],
                    lnl[:, lt, :], kplT[:, ts(lt, 128)],
                    latTl[:, :, ts(lt, 128)])
            # only latT + k_pe^T are gathered: AV now consumes lv (derived
            # locally from latT), so the [t,k] latent never crosses cores
            cc_in = dram.tile([128, 1280], bf16)
            nc.gpsimd.dma_start(out=cc_in[:, ds(0, 1024)], in_=latTl[:])
            nc.gpsimd.dma_start(out=cc_in[:, ds(1024, 256)], in_=kplT[:])
            if os.environ.get("KERNEL_SIM_NO_CC"):
                # timing-only stand-in for TimelineSim (no collective
                # support there). Numerically wrong — never for real runs.
                cc_out = dram.tile([8, 128, 1280], bf16)
                nc.gpsimd.dma_start(out=cc_out[0], in_=cc_in[:])
            else:
                cc_out = dram.tile([8, 128, 1280], bf16, addr_space="Shared")
                nc.gpsimd.collective_compute(
                    "AllGather", mybir.AluOpType.bypass,
                    replica_groups=[list(range(NCORES))],
                    ins=[cc_in.opt()], outs=[cc_out.opt()])

            # block-0 tiles 0..3, local on every core (identical values)
            for ttl in range(4):
                kv_tile(
                    (lambda kt, ttl=ttl:
                     a0chunks[kt // 4][:, kt % 4, ds(ttl * 128, 128)]),
                    csl_sb[:, ttl, :],
                    lnb[:, ttl, :], kpeT_sb[:, ts(ttl, 128)],
                    latT_sb[:, :, ts(ttl, 128)])

            # scatter gathered tiles 4..15 (slots 0..5) into shared state;
            # slots 0,1 feed block-1 attention soonest -> quiet SP queue
            for s in range(6):
                t0 = 4 + 2 * s
                if s < 2:
                    eng = nc.sync
                else:
                    eng = nc.gpsimd if s % 2 == 0 else nc.scalar
                eng.dma_start(out=latT_sb[:, :, ds(128 * t0, 256)],
                              in_=cc_out[s, :, ds(0, 1024)])
                eng.dma_start(out=kpeT_sb[:, ds(128 * t0, 256)],
                              in_=cc_out[s, :, ds(1024, 256)])

            # ---- phase E: o_proj for a finished block ----
            def phase_e(je):
                for stl in range(4):
                    st = 4 * je + stl
                    for half in range(2):
                        ob = outp.tile([128, 1024], bf16, tag="ob")
                        for hh in range(2):
                            hb = 2 * half + hh
                            p_o = pmm.tile([128, 512], f32, tag="mm")
                            nc.tensor.matmul(
                                p_o, lhsT=ovT_sb[:, 0, ts(st, 128)],
                                rhs=wo_sb[:, 0, ts(hb, 512)],
                                start=True, stop=False)
                            nc.tensor.matmul(
                                p_o, lhsT=ovT_sb[:, 1, ts(st, 128)],
                                rhs=wo_sb[:, 1, ts(hb, 512)],
                                start=False, stop=True)
                            # alternate copy engine so neither DVE nor ACT
                            # serializes the store pipeline
                            if hh == 0:
                                nc.vector.tensor_copy(ob[:, ts(hh, 512)], p_o)
                            else:
                                nc.scalar.copy(ob[:, ts(hh, 512)], p_o)
                        # alternate DMA queues (Pool vs SP) so stores overlap
                        # each other and the x prefetches
                        dma_eng = nc.gpsimd if half == 0 else nc.sync
                        dma_eng.dma_start(
                            out=out[ts(st, 128), ds(1024 * half, 1024)],
                            in_=ob)

            # ---- phase C: q projections for block j ----
            def phase_c(j, xtile):
                # raw q_nope^T [d, s] is the scores moving operand directly
                # (no absorb stage — keys are un-absorbed instead)
                qls = []
                for h in range(2):
                    p_qn = pmm.tile([128, 512], f32, tag="mm")
                    for kt in range(16):
                        nc.tensor.matmul(
                            p_qn, lhsT=wqn_sb[:, kt, ts(h, 128)],
                            rhs=xtile(kt, 0, 512),
                            start=(kt == 0), stop=(kt == 15))
                    qn_bf = work.tile([128, 512], bf16, tag=f"qn{h}",
                                      bufs=3)
                    if h == 0:
                        nc.scalar.copy(qn_bf, p_qn)
                    else:
                        nc.vector.tensor_copy(qn_bf, p_qn)
                    qls.append(qn_bf)
                # q_pe^T directly: stationary wqp chunk [hid, 128=(h0|h1 d)],
                # moving x^T [hid, 512 s] -> [d, s] heads stacked on
                # partitions; rope via partition-shifted DVE ops on cs^T
                qpe_rot = work.tile([128, 512], bf16, tag="qpr", bufs=3)
                p_qp = pmm.tile([128, 512], f32, tag="mm")
                for kt in range(16):
                    nc.tensor.matmul(
                        p_qp, lhsT=wqp_sb[:, kt, :], rhs=xtile(kt, 0, 512),
                        start=(kt == 0), stop=(kt == 15))
                scol = ds(512 * j, 512)
                ckq = cst_sb[:, 0, scol]
                skq = cst_sb[:, 1, scol]
                t1q = scr.tile([128, 512], f32, tag="qt1", bufs=1)
                t2q = scr.tile([128, 512], f32, tag="qt2", bufs=1)
                qdeps = [nc.vector.tensor_mul(t1q, p_qp, ckq)]
                for h2 in range(2):
                    b = 64 * h2
                    qdeps.append(nc.vector.tensor_mul(
                        t2q[ds(b, 32), :], p_qp[ds(b + 32, 32), :],
                        skq[ds(b, 32), :]))
                    qdeps.append(nc.vector.tensor_mul(
                        t2q[ds(b + 32, 32), :], p_qp[ds(b, 32), :],
                        skq[ds(b + 32, 32), :]))
                if j == 0:
                    for ii in qdeps:
                        add_dep_helper(ii.ins, cst_touch_inst.ins,
                                       sync=False,
                                       reason="cst first-touch order")
                nc.vector.tensor_add(qpe_rot, t1q, t2q)
                return qls, qpe_rot

            # ---- phase D: attention for s-block j, each head ----
            def phase_d(j, qls, qpe_rot):
                n_t = 4 * (j + 1)
                # un-absorbed values and keys for this block's 4 new tiles:
                # lv[t, v] = sum_k latT[k, t] * vc[k, v]
                # khT[d, t] = sum_k kc[k, d] * latT[k, t]
                for h in range(2):
                    for tl in range(4):
                        tt = 4 * j + tl
                        p_lv = psmall.tile([128, 128], f32, tag="small")
                        for kk in range(4):
                            nc.tensor.matmul(
                                p_lv, lhsT=latT_sb[:, kk, ts(tt, 128)],
                                rhs=vc_sb[:, h, kk, :],
                                start=(kk == 0), stop=(kk == 3))
                        if tl % 2 == 0:
                            nc.scalar.copy(lv_sb[:, h, tt, :], p_lv)
                        else:
                            nc.vector.tensor_copy(lv_sb[:, h, tt, :], p_lv)
                        p_kh = psmall.tile([128, 128], f32, tag="small")
                        for kk in range(4):
                            nc.tensor.matmul(
                                p_kh, lhsT=kc_sb[:, kk, h, :],
                                rhs=latT_sb[:, kk, ts(tt, 128)],
                                start=(kk == 0), stop=(kk == 3))
                        if tl % 2 == 0:
                            nc.vector.tensor_copy(khT_sb[:, h, tt, :], p_kh)
                        else:
                            nc.scalar.copy(khT_sb[:, h, tt, :], p_kh)
                for h in range(2):
                    ql = qls[h]
                    atiles = []
                    for tt in range(n_t):
                        # diagonal tiles: columns s < t are fully masked —
                        # skip them in matmuls/exp (c0 = first live col)
                        c0 = 128 * (tt - 4 * j) if tt >= 4 * j else 0
                        p_sc = pmm.tile([128, 512], f32, tag="mm")
                        nc.tensor.matmul(
                            p_sc[:, c0:512],
                            lhsT=khT_sb[:, h, tt, :],
                            rhs=ql[:, c0:512],
                            start=True, stop=False)
                        nc.tensor.matmul(
                            p_sc[:, c0:512],
                            lhsT=kpeT_sb[ds(64 * h, 64), ts(tt, 128)],
                            rhs=qpe_rot[ds(64 * h, 64), c0:512],
                            start=False, stop=True)
                        at = attnp.tile([128, 512], bf16, tag="attn")
                        nc.scalar.activation(
                            out=at[:, c0:512], in_=p_sc[:, c0:512],
                            func=mybir.ActivationFunctionType.Exp, scale=SCALE)
                        if tt >= 4 * j:
                            # partial mask inside the first live 128 cols:
                            # keep where (s' - p) >= 0 in-tile
                            nc.gpsimd.affine_select(
                                out=at[:, c0:c0 + 128],
                                in_=at[:, c0:c0 + 128],
                                compare_op=mybir.AluOpType.is_ge,
                                fill=0.0, base=0,
                                channel_multiplier=-1, pattern=[[1, 128]])
                        atiles.append(at)
                    # denominator: chained ones-matmul sums the partition
                    # dim AND broadcasts to all 128 partitions in one bank
                    p_den = pden.tile([128, 512], f32, tag="den")
                    for tt in range(n_t):
                        c0 = 128 * (tt - 4 * j) if tt >= 4 * j else 0
                        nc.tensor.matmul(
                            p_den[:, c0:512], lhsT=ones_bf,
                            rhs=atiles[tt][:, c0:512],
                            start=(tt == 0), stop=(tt == n_t - 1))
                    rb_sb = scr.tile([128, 512], f32, tag="scr")
                    nc.vector.reciprocal_approx_fast(rb_sb, p_den)
                    # AV directly in value space: out_v^T[v, s] via the
                    # un-absorbed lv — one chunk instead of four + vc
                    p_ov = pmm.tile([128, 512], f32, tag="mm")
                    for tt in range(n_t):
                        c0 = 128 * (tt - 4 * j) if tt >= 4 * j else 0
                        nc.tensor.matmul(
                            p_ov[:, c0:512], lhsT=lv_sb[:, h, tt, :],
                            rhs=atiles[tt][:, c0:512],
                            start=(tt == 0), stop=(tt == n_t - 1))
                    nc.vector.tensor_mul(
                        ovT_sb[:, h, ts(j, 512)], p_ov, rb_sb)

            # ---- block loop, software-pipelined: C runs one block ahead
            # of D; block-0 attention needs no gather (local tiles), and
            # the gather lands well before block-1 attention ----
            cq = {}
            for j in range(NB):
                # phase A: load x^T block (block 0 already loaded above)
                if j == 0:
                    xchunks = a0chunks
                else:
                    xchunks = []
                    for c in range(4):
                        xc = xpool.tile([128, 4, 512], bf16, tag="xt")
                        # alternate Pool/ACT DMA queues: x shares Pool with
                        # the gather bounce+scatter and half the stores
                        eng = nc.scalar if c % 2 == 1 else nc.gpsimd
                        eng.dma_start(out=xc, in_=xt[j, :, ts(c, 4), :])
                        xchunks.append(xc)

                def xtile(kt, col0, ncol, xchunks=xchunks):
                    return xchunks[kt // 4][:, kt % 4, ds(col0, ncol)]

                cq[j] = phase_c(j, xtile)
                if j >= 3:
                    phase_e(j - 3)
                if j >= 2:
                    phase_d(j - 2, *cq.pop(j - 2))
            phase_e(NB - 3)
            phase_d(NB - 2, *cq.pop(NB - 2))
            phase_e(NB - 2)
            phase_d(NB - 1, *cq.pop(NB - 1))
            phase_e(NB - 1)
    nc.compile()
    return nc


def _prep_inputs(hidden_states, cos, sin, w_q, w_kv_a, kv_norm_w, kc, vc, w_o,
                 s_len):
    """Host-side sharding + layout. Returns list of 8 per-core input dicts."""
    f32 = np.float32
    x = np.asarray(hidden_states, f32)[0][:s_len]  # [S, HID]
    cos = np.asarray(cos, f32)[:s_len]
    sin = np.asarray(sin, f32)[:s_len]
    w_q = np.asarray(w_q, f32).reshape(HID, H, NOPE + ROPE)
    w_kv_a = np.asarray(w_kv_a, f32)
    w = np.asarray(kv_norm_w, f32)
    kc = np.asarray(kc, f32)
    vc = np.asarray(vc, f32)
    w_o = np.asarray(w_o, f32).reshape(H, VH, HID)

    NB = s_len // 512

    # x^T in [j, p, a, s'] blocks: x^T[hid=a*128+p, s=j*512+s']
    xT = np.ascontiguousarray(x.T).astype(BF16)    # [HID, S]
    xt_b = np.ascontiguousarray(
        xT.reshape(16, 128, NB, 512).transpose(2, 1, 0, 3))

    wkv_b = np.ascontiguousarray(
        w_kv_a.astype(BF16).reshape(16, 128, 576).transpose(1, 0, 2))

    # rope tables: sin_eff has its first half negated
    sin_eff = np.concatenate([-sin[:, :32], sin[:, 32:]], axis=1)
    # transposed tables duplicated across both 64-partition halves
    cosT = np.ascontiguousarray(cos.T)      # [64, S]
    sinT = np.ascontiguousarray(sin_eff.T)  # [64, S]
    cst_b = np.ascontiguousarray(
        np.stack([np.concatenate([cosT, cosT], 0),
                  np.concatenate([sinT, sinT], 0)], axis=1)).astype(BF16)
    # per-t-tile rope table [p, tt, 128]
    cs_full = np.ascontiguousarray(
        np.concatenate([cos, sin_eff], axis=1)
        .reshape(s_len // 128, 128, 128).transpose(1, 0, 2))
    xT3 = xT.reshape(16, 128, s_len)

    in_maps = []
    for c in range(NCORES):
        hs = [HPC * c, HPC * c + 1]
        wqn_c = np.ascontiguousarray(
            w_q[:, hs, :NOPE].reshape(16, 128, 256).transpose(1, 0, 2)
        ).astype(BF16)
        wqp_c = np.ascontiguousarray(
            w_q[:, hs, NOPE:].reshape(16, 128, 128).transpose(1, 0, 2)
        ).astype(BF16)
        # kc^T chunks [k-part, kk, h, d] (kv_norm folded in)
        kc_c = np.ascontiguousarray(
            (kc[hs] * w[None, None, :]).transpose(2, 0, 1)
            .reshape(4, 128, 2, 128).transpose(1, 0, 2, 3)).astype(BF16)
        vc_c = np.ascontiguousarray(
            (vc[hs] * w[None, :, None]).reshape(2, 4, 128, 128)
            .transpose(2, 0, 1, 3)).astype(BF16)
        wo_c = np.ascontiguousarray(w_o[hs].transpose(1, 0, 2)).astype(BF16)
        # hybrid sharding: cores 0..5 own tiles (4+2c, 5+2c); cores 6,7
        # compute unused dummies (their gather slots are ignored)
        t0 = 4 + 2 * c if c < 6 else 4
        xp_c = np.ascontiguousarray(
            xT3[:, :, 128 * t0:128 * t0 + 256].transpose(1, 0, 2))
        csl_c = np.ascontiguousarray(np.concatenate(
            [cs_full[:, 0:4, :], cs_full[:, t0:t0 + 2, :]], axis=1))
        in_maps.append({
            "xt": xt_b, "xp": xp_c, "wkv": wkv_b,
            "wqn": wqn_c, "wqp": wqp_c, "kcp": kc_c, "vcp": vc_c, "wo": wo_c,
            "csl": csl_c, "cst": cst_b,
        })
    return in_maps


def run(inputs, trace=False, s_len=S):
    """Returns (full_output [1,S,HID] f32, exec_time_ns or None, trace_path)."""
    from concourse import bass_utils

    if s_len not in _CACHE:
        _CACHE[s_len] = _build_nc(s_len)
    nc = _CACHE[s_len]
    in_maps = _prep_inputs(**inputs, s_len=s_len)
    res = bass_utils.run_bass_kernel_spmd(
        nc, in_maps, core_ids=list(range(NCORES)), trace=False)
    acc = np.zeros((s_len, HID), np.float64)
    for r in res.results:
        acc += r["out"].astype(np.float64)
    out = acc.astype(np.float32)[None]
    return out, None, None


def _pjrt_callable(nc, n_cores):
    """Build a jax-jitted SPMD callable for `nc` (no donation: every output
    element is written by the kernel, so uninit result buffers are fine)."""
    import jax
    from jax.sharding import Mesh, PartitionSpec, NamedSharding
    from jax.experimental.shard_map import shard_map
    from concourse import bass2jax, mybir

    bass2jax.install_neuronx_cc_hook()
    part_name = nc.partition_id_tensor.name if nc.partition_id_tensor else None
    in_names, out_names, out_avals, zero_outs = [], [], [], []
    for alloc in nc.m.functions[0].allocations:
        if not isinstance(alloc, mybir.MemoryLocationSet):
            continue
        name = alloc.memorylocations[0].name
        if alloc.kind == "ExternalInput":
            if name != part_name:
                in_names.append(name)
        elif alloc.kind == "ExternalOutput":
            out_names.append(name)
            out_avals.append(jax.core.ShapedArray(
                tuple(alloc.tensor_shape), mybir.dt.np(alloc.dtype)))
            zero_outs.append(np.zeros(
                tuple(alloc.tensor_shape), mybir.dt.np(alloc.dtype)))
    n_params = len(in_names)
    all_names = in_names + out_names
    if part_name is not None:
        all_names = all_names + [part_name]

    def _body(*args):
        operands = list(args)
        if part_name is not None:
            operands.append(bass2jax.partition_id_tensor())
        outs = bass2jax._bass_exec_p.bind(
            *operands,
            out_avals=tuple(out_avals),
            in_names=tuple(all_names),
            out_names=tuple(out_names),
            lowering_input_output_aliases=(),
            sim_require_finite=True,
            sim_require_nnan=True,
            nc=nc,
        )
        return tuple(outs)

    devices = jax.devices()[:n_cores]
    mesh = Mesh(np.asarray(devices), ("core",))
    spec = PartitionSpec("core")
    donate = tuple(range(n_params, n_params + len(out_names)))
    sharded = jax.jit(
        shard_map(_body, mesh=mesh,
                  in_specs=(spec,) * (n_params + len(out_names)),
                  out_specs=(spec,) * len(out_names), check_rep=False),
        donate_argnums=donate, keep_unused=True)
    sharding = NamedSharding(mesh, spec)
    return sharded, in_names, out_names, zero_outs, sharding


def timed_run(inputs, iters=6, s_len=S):
    """Run on HW with device-resident inputs; return (out, per-call walls)."""
    import jax
    import time

    if s_len not in _CACHE:
        _CACHE[s_len] = _build_nc(s_len)
    nc = _CACHE[s_len]
    in_maps = _prep_inputs(**inputs, s_len=s_len)
    sharded, in_names, out_names, zero_outs, sharding = _pjrt_callable(
        nc, NCORES)
    concat_in = [
        jax.device_put(
            np.concatenate([np.asarray(in_maps[c][n]) for c in range(NCORES)],
                           axis=0), sharding)
        for n in in_names
    ]
    def zeros_set():
        return [
            jax.device_put(
                np.zeros((NCORES * z.shape[0], *z.shape[1:]), z.dtype),
                sharding)
            for z in zero_outs
        ]

    # donation consumes each zero set, so pre-stage one per call
    sets = [zeros_set() for _ in range(iters + 1)]
    out_arrs = jax.block_until_ready(sharded(*concat_in, *sets[0]))
    walls = []
    for it in range(iters):
        t0 = time.perf_counter()
        out_arrs = jax.block_until_ready(sharded(*concat_in, *sets[it + 1]))
        walls.append(time.perf_counter() - t0)
    full = np.asarray(out_arrs[0]).reshape(NCORES, s_len, HID)
    out = full.astype(np.float64).sum(0).astype(np.float32)[None]
    return out, walls


def async_slope(inputs, ks=(1, 9), s_len=S):
    """Dispatch K calls without blocking, block once; slope over K gives
    per-exec time with the axon RPC overhead pipelined away (if the
    device queue overlaps dispatch)."""
    import jax
    import time

    if s_len not in _CACHE:
        _CACHE[s_len] = _build_nc(s_len)
    nc = _CACHE[s_len]
    in_maps = _prep_inputs(**inputs, s_len=s_len)
    sharded, in_names, out_names, zero_outs, sharding = _pjrt_callable(
        nc, NCORES)
    concat_in = [
        jax.device_put(
            np.concatenate([np.asarray(in_maps[c][n]) for c in range(NCORES)],
                           axis=0), sharding)
        for n in in_names
    ]

    def zeros_set():
        return [
            jax.device_put(
                np.zeros((NCORES * z.shape[0], *z.shape[1:]), z.dtype),
                sharding)
            for z in zero_outs
        ]

    jax.block_until_ready(sharded(*concat_in, *zeros_set()))  # warm
    times = {}
    for k in ks:
        # ping-pong: output buffers (same shape/sharding as the donated
        # zero inputs) feed call i+2, so no host->device transfers and
        # the device queue runs the chain back-to-back
        outs = [sharded(*concat_in, *zeros_set()),
                sharded(*concat_in, *zeros_set())]
        jax.block_until_ready(outs)
        t0 = time.perf_counter()
        for i in range(k):
            outs.append(sharded(*concat_in, *outs[-2]))
        jax.block_until_ready(outs[-1])
        times[k] = time.perf_counter() - t0
    k0, k1 = ks
    slope = (times[k1] - times[k0]) / (k1 - k0)
    return slope, times


_TRIV = {}


def trivial_walls(iters=6):
    """Dispatch-overhead floor: time a near-empty 8-core bass kernel."""
    import jax
    import time
    import concourse.tile as tile
    from concourse import bacc, mybir

    if "nc" not in _TRIV:
        nc = bacc.Bacc()
        tin = nc.declare_dram_parameter("tin", [128, 128], mybir.dt.float32,
                                        isOutput=False)
        tout = nc.declare_dram_parameter("tout", [128, 128], mybir.dt.float32,
                                         isOutput=True)
        with tile.TileContext(nc) as tc:
            with tc.tile_pool(name="p", bufs=1) as p:
                t = p.tile([128, 128], mybir.dt.float32)
                nc.sync.dma_start(out=t, in_=tin[:])
                nc.sync.dma_start(out=tout[:], in_=t)
        nc.compile()
        _TRIV["nc"] = nc
    nc = _TRIV["nc"]
    sharded, in_names, out_names, zero_outs, sharding = _pjrt_callable(
        nc, NCORES)
    x = jax.device_put(np.zeros((NCORES * 128, 128), np.float32), sharding)

    def z():
        return jax.device_put(
            np.zeros((NCORES * 128, 128), np.float32), sharding)

    zs = [z() for _ in range(iters + 1)]
    jax.block_until_ready(sharded(x, zs[0]))
    walls = []
    for it in range(iters):
        t0 = time.perf_counter()
        jax.block_until_ready(sharded(x, zs[it + 1]))
        walls.append(time.perf_counter() - t0)
    return walls


def kernel(**inputs):
    out, _, _ = run(inputs, trace=False)
    return out

